# revision 10
# baseline (speedup 1.0000x reference)
"""ObjectAttentionBlock2D TRN2 kernel.

Reference computation (per batch b):
    xf    = x[b].reshape(C, N)                  # C=512, N=128*128=16384
    pf    = proxy[b,:,:,0]                      # [C, K], K=64
    query = Wq @ xf + bq                        # [Ck=256, N]
    keym  = Wk @ pf + bk                        # [Ck, K]
    value = (Wv @ pf + bv).T                    # [K, Cv=256]
    sim   = softmax_k(query.T @ keym / 16)      # [N, K]
    ctx   = sim @ value                         # [N, Cv]
    out   = Wo @ ctx.T + bo                     # [C, N]

Sharding: data-parallel over batch. B=8 batches -> 8 NeuronCores, one image
per core, no collectives.

Algebraic folds (all rank-K, K=64):
  M     = Wq^T @ keym            [C, K]  -> sim = M^T x      (Q proj folded)
  sbias = (bq/16)^T @ keym       [K, 1]  -> rides in exp's bias slot
  WVT'  = (Wo @ value^T)^T + bo  [K, C]  -> out rows = WVT'^T e * r
The +bo fold works because softmax rows sum to 1: with unnormalized
e = exp(logits) and r = 1/sum_k e,  r * (WVT+bo)^T e = WVT^T en + bo.

The whole pipeline is DMA-bound (the cost model serializes all DMA traffic
on one shared 360 B/ns device): x-in 16.8MB + out 16.8MB ~= 93us. Everything
else is arranged to fit underneath:
  - out is written fp16 (host upcasts); rel err ~8e-4 vs 2e-2 budget.
  - out matmuls are TRANSPOSED ([pixel, channel] psum, lhsT = e-slice,
    rhs = WVT'): softmax denominators land per-PARTITION, so the
    normalization multiply fuses into the psum->sbuf convert copies
    (ACT activation Copy with scale=r, DVE tensor_scalar_mul) - no
    broadcast matmul, no separate normalize pass.
  - den[n] = sum_k e[k,n] comes from 4 free-size-1 matmuls (lhsT=e chunk,
    rhs=ones column) - negligible PE time.
  - F=512 pixel tiles amortize fixed per-op overheads; x is DMAd in
    1024-column chunks (2KB descriptors) via Pool/SWDGE, out on SP/HWDGE.
Host writes back out^T [N, C] fp16; kernel() transposes + upcasts.

Per-2-tile engine budget (ns, cost model): DMA 2912 | PE ~1750 |
ACT 1836 | DVE ~1450 | Pool ~1170. TimelineSim exec ~ 100us/core.
"""

import numpy as np

import concourse.bacc as bacc
import concourse.mybir as mybir
import concourse.tile as tile
from concourse import bass_utils

F32 = mybir.dt.float32
F32R = mybir.dt.float32r
F16 = mybir.dt.float16

B, C, H, W = 8, 512, 128, 128
N = H * W                    # 16384 pixels per image
CK, CV, K = 256, 256, 64
P = 128                      # SBUF partitions
F = 512                      # pixel-tile width
NT = N // F                  # 32 tiles
FD = 1024                    # x DMA chunk width (2 tiles)
CI_CH = C // P               # 4 contraction chunks over C
Q_CH = CK // P               # 2 chunks over Ck
V_CH = CV // P               # 2 chunks over Cv
NC_CH = F // P               # 4 pixel chunks per tile
SCALE = CK ** -0.5           # 1/16

_CACHED = None


def _build():
    nc = bacc.Bacc("TRN2", target_bir_lowering=False, debug=False)

    X = nc.dram_tensor("x", [C, N], F16, kind="ExternalInput").ap()
    # pack16[c, :] = [pf(64) | wkT(256) | wvT(256)] in fp16
    PACK16 = nc.dram_tensor("pack16", [C, 576], F16, kind="ExternalInput").ap()
    WQ = nc.dram_tensor("wq", [CK, C], F16, kind="ExternalInput").ap()
    WO16 = nc.dram_tensor("wo", [CV, C], F16, kind="ExternalInput").ap()
    # crow = [bk(256) | bv(256) | ones(256) | bo(512)] as one row
    CROW = nc.dram_tensor("crow", [1, 1280], F32, kind="ExternalInput").ap()
    BQS16 = nc.dram_tensor("bqs16", [P, 2], F16, kind="ExternalInput").ap()
    # out^T: row n holds all 512 output channels of pixel n, affine uint8:
    # u8 = rne(out*s + 128). The scale s rides in via host-prescaled Wo/bo
    # (WVT'' = s*WVT'), so the compiled module is identical on every core.
    OUTT = nc.dram_tensor("out", [N, C], mybir.dt.uint8, kind="ExternalOutput").ap()

    x_r = X.rearrange("(co p) n -> p co n", p=P)                   # [128, 4, N]
    out_r = OUTT.rearrange("(t c p) o -> p t c o", c=NC_CH, p=P)   # [128, 32, 4, 512]

    with tile.TileContext(nc) as tc:
        with tc.tile_pool(name="const", bufs=1) as cp:
            # tiny constant rows FIRST: the whole setup-matmul chain waits on
            # crow (bias rows), so it must not queue behind the big weights
            crow = cp.tile([1, 1280], F32R)
            nc.scalar.dma_start(crow, CROW.bitcast(F32R))
            bk_row = crow[:, 0:CK]
            bv_row = crow[:, CK:CK + CV]
            ones_row = crow[:, 512:768]
            bo_row = crow[:, 768:1280]
            bqs = cp.tile([P, 2], F16)
            nc.scalar.dma_start(bqs, BQS16)
            ones_col = cp.tile([K, 1], F16)
            nc.vector.memset(ones_col, 1.0)
            b128 = cp.tile([P, 1], F32)
            nc.vector.memset(b128, 128.0)
            pack = cp.tile([P, CI_CH, 576], F16)
            nc.sync.dma_start(pack, PACK16.rearrange("(co p) q -> p co q", p=P))
            pf = pack[:, :, 0:K]
            wk = pack[:, :, K:K + CK]
            wv = pack[:, :, K + CK:K + CK + CV]
            wq = cp.tile([P, Q_CH, C], F16)
            nc.sync.dma_start(wq, WQ.rearrange("(qo p) c -> p qo c", p=P))
            wo = cp.tile([P, V_CH, C], F16)
            nc.sync.dma_start(wo, WO16.rearrange("(vo p) o -> p vo o", p=P))

            keym = cp.tile([P, Q_CH, K], F16)    # [q-part, q-chunk, k]
            v2sb = cp.tile([P, V_CH, K], F16)    # value[k,v] as [v-part, vo, k]
            wvt = cp.tile([K, C], F16)           # WVT'[k,o] = (Wo@value^T)^T + bo
            msim = cp.tile([P, CI_CH, K], F16)   # M[c,k] = sum_q Wq[q,c]*keym[q,k]
            sbias = cp.tile([K, 1], F32)         # sum_q (bq[q]/16)*keym[q,k]

            # ---- one-time setup: keym, value, WVT', M, sbias
            with tc.tile_pool(name="setup_ps", bufs=1, space="PSUM") as sps:
                kps = sps.tile([P, Q_CH, K], F32)
                for qi in range(Q_CH):
                    for ci in range(CI_CH):
                        nc.tensor.matmul(
                            kps[:, qi, :],
                            wk[:, ci, qi * P:(qi + 1) * P],
                            pf[:, ci, :],
                            start=(ci == 0), stop=False,
                        )
                    # += bk[q] * ones[k]
                    nc.tensor.matmul(
                        kps[:, qi, :],
                        bk_row[:, qi * P:(qi + 1) * P],
                        ones_row[:, :K],
                        start=False, stop=True,
                    )
                nc.vector.tensor_copy(keym, kps)

                v2ps = sps.tile([P, V_CH, K], F32)
                for vi in range(V_CH):
                    for ci in range(CI_CH):
                        nc.tensor.matmul(
                            v2ps[:, vi, :],
                            wv[:, ci, vi * P:(vi + 1) * P],
                            pf[:, ci, :],
                            start=(ci == 0), stop=False,
                        )
                    nc.tensor.matmul(
                        v2ps[:, vi, :],
                        bv_row[:, vi * P:(vi + 1) * P],
                        ones_row[:, :K],
                        start=False, stop=True,
                    )
                nc.vector.tensor_copy(v2sb, v2ps)

                wvtps = sps.tile([K, C], F32)
                for vi in range(V_CH):
                    nc.tensor.matmul(
                        wvtps, v2sb[:, vi, :], wo[:, vi, :],
                        start=(vi == 0), stop=False,
                    )
                # += ones[k] * bo[o]  (valid because softmax rows sum to 1)
                nc.tensor.matmul(
                    wvtps, ones_row[:, :K], bo_row,
                    start=False, stop=True,
                )
                nc.vector.tensor_copy(wvt, wvtps)

                mps = sps.tile([P, CI_CH, K], F32)
                for ci in range(CI_CH):
                    for qi in range(Q_CH):
                        nc.tensor.matmul(
                            mps[:, ci, :],
                            wq[:, qi, ci * P:(ci + 1) * P],
                            keym[:, qi, :],
                            start=(qi == 0), stop=(qi == Q_CH - 1),
                        )
                nc.vector.tensor_copy(msim, mps)

                sbps = sps.tile([K, 1], F32)
                for qi in range(Q_CH):
                    nc.tensor.matmul(
                        sbps, keym[:, qi, :], bqs[:, qi:qi + 1],
                        start=(qi == 0), stop=(qi == Q_CH - 1),
                    )
                nc.vector.tensor_copy(sbias, sbps)

            # ---- steady-state pipeline over 32 tiles of F=512 pixels
            # Software-pipelined one tile deep: iteration t emits the
            # sim/exp front-end for tile t and the den/out/copy/DMA back-end
            # for tile t-1, so no engine's in-order stream ever waits on a
            # same-tile cross-engine chain (ACT: exp(t) precedes copies(t-1)).
            with (
                tc.tile_pool(name="xin", bufs=8) as xp,
                tc.tile_pool(name="esb", bufs=4) as ep,
                tc.tile_pool(name="rsb", bufs=3) as rp,
                tc.tile_pool(name="outsb", bufs=5) as osp,
                tc.tile_pool(name="simps", bufs=2, space="PSUM") as simp,
                tc.tile_pool(name="denps", bufs=2, space="PSUM") as denp,
                tc.tile_pool(name="outps", bufs=4, space="PSUM") as outp,
            ):
                x_t = None
                e_p = None
                for t in range(NT + 1):
                    if t < NT:
                        n0 = t * F
                        if t % 2 == 0:
                            x_t = xp.tile([P, CI_CH, FD], F16, tag="x")
                            nc.gpsimd.dma_start(x_t, x_r[:, :, n0:n0 + FD])
                        off = (t % 2) * F

                        # sim[k, n] = M^T x (Q projection folded into M)
                        sim = simp.tile([K, F], F32, tag="sim")
                        for ci in range(CI_CH):
                            nc.tensor.matmul(
                                sim, msim[:, ci, :], x_t[:, ci, off:off + F],
                                start=(ci == 0), stop=(ci == CI_CH - 1),
                            )
                        e = ep.tile([K, F], F16, tag="e")
                        nc.scalar.activation(
                            e, sim, mybir.ActivationFunctionType.Exp,
                            scale=SCALE, bias=sbias,
                        )

                    if e_p is not None:
                        tp = t - 1
                        # den[n] (pixels on partitions): lhsT = e slice
                        den = denp.tile([P, NC_CH], F32, tag="den")
                        for c in range(NC_CH):
                            nc.tensor.matmul(
                                den[:, c:c + 1],
                                e_p[:, c * P:(c + 1) * P], ones_col,
                                start=True, stop=True,
                            )
                        r_sb = rp.tile([P, NC_CH], F32, tag="r")
                        with nc.allow_low_precision(reason="f32 recip on DVE"):
                            nc.vector.reciprocal(r_sb, den)

                        # out^T chunks [pixel, channel]; normalization (x r)
                        # and affine uint8 quantization (+128, rne) fuse into
                        # the psum->sbuf convert copies
                        out_sb = osp.tile([P, NC_CH, C], mybir.dt.uint8, tag="osb")
                        for c in range(NC_CH):
                            op = outp.tile([P, C], F32, tag="op")
                            nc.tensor.matmul(
                                op, e_p[:, c * P:(c + 1) * P], wvt,
                                start=True, stop=True,
                            )
                            if c < 2:
                                nc.scalar.activation(
                                    out_sb[:, c, :], op,
                                    mybir.ActivationFunctionType.Identity,
                                    bias=b128, scale=r_sb[:, c:c + 1],
                                )
                            else:
                                nc.vector.tensor_scalar(
                                    out_sb[:, c, :], op,
                                    r_sb[:, c:c + 1], 128.0,
                                    op0=mybir.AluOpType.mult,
                                    op1=mybir.AluOpType.add,
                                )
                        nc.sync.dma_start(out_r[:, tp, :, :], out_sb)
                    e_p = e if t < NT else None

    nc.compile()
    return nc


def _get_nc():
    global _CACHED
    if _CACHED is None:
        _CACHED = _build()
    return _CACHED


def kernel(x, proxy, Wq, bq, Wk, bk, Wv, bv, Wo, bo, **run_kwargs):
    nc = _get_nc()

    w16 = np.concatenate(
        [np.asarray(Wk).T, np.asarray(Wv).T], axis=1
    ).astype(np.float16)
    wo32 = np.asarray(Wo, np.float32)
    wv16 = np.asarray(Wv).astype(np.float16).astype(np.float32)
    bo32 = np.asarray(bo, np.float32)
    shared = {
        "wq": np.ascontiguousarray(Wq).astype(np.float16),
        "bqs16": np.ascontiguousarray(
            (np.asarray(bq, np.float32) * SCALE).reshape(2, P).T
        ).astype(np.float16),
    }
    in_maps = []
    scales = []
    for b in range(B):
        m = dict(shared)
        m["x"] = np.ascontiguousarray(x[b]).reshape(C, N).astype(np.float16)
        pf16 = np.asarray(proxy[b, :, :, 0]).astype(np.float16)
        m["pack16"] = np.ascontiguousarray(np.concatenate([pf16, w16], axis=1))
        # uint8 scale: out rows are convex combinations of WVT' columns, so
        # max|WVT' + bo| bounds |out|. Replicate the device's fp16 setup math.
        v216 = (wv16 @ pf16.astype(np.float32)
                + np.asarray(bv, np.float32)[:, None]).astype(np.float16)
        wvtp = wo32 @ v216.astype(np.float32) + bo32[:, None]
        s = 126.0 / (np.abs(wvtp).max() * 1.01)
        scales.append(s)
        crow = np.concatenate(
            [np.asarray(bk, np.float32).reshape(1, CK),
             np.asarray(bv, np.float32).reshape(1, CV),
             np.ones((1, 256), np.float32),
             (s * bo32).reshape(1, C)], axis=1)
        m["crow"] = np.ascontiguousarray(crow)
        m["wo"] = np.ascontiguousarray(s * wo32.T).astype(np.float16)
        in_maps.append(m)

    res = bass_utils.run_bass_kernel_spmd(
        nc, in_maps, core_ids=list(range(B)), **run_kwargs
    )
    out = np.stack(
        [(res.results[b]["out"].astype(np.float32) - 128.0).T / scales[b]
         for b in range(B)], axis=0
    )
    if run_kwargs:
        kernel.last_results = res
    return out.reshape(B, C, H, W)


# revision 11
# speedup vs baseline: 1.1043x; 1.1043x over previous
"""ObjectAttentionBlock2D TRN2 kernel.

Reference computation (per batch b):
    xf    = x[b].reshape(C, N)                  # C=512, N=128*128=16384
    pf    = proxy[b,:,:,0]                      # [C, K], K=64
    query = Wq @ xf + bq                        # [Ck=256, N]
    keym  = Wk @ pf + bk                        # [Ck, K]
    value = (Wv @ pf + bv).T                    # [K, Cv=256]
    sim   = softmax_k(query.T @ keym / 16)      # [N, K]
    ctx   = sim @ value                         # [N, Cv]
    out   = Wo @ ctx.T + bo                     # [C, N]

Sharding: data-parallel over batch. B=8 batches -> 8 NeuronCores, one image
per core, no collectives.

Algebraic folds (all rank-K, K=64):
  M     = Wq^T @ keym            [C, K]  -> sim = M^T x      (Q proj folded)
  sbias = (bq/16)^T @ keym       [K, 1]  -> rides in exp's bias slot
  WVT'  = (Wo @ value^T)^T + bo  [K, C]  -> out rows = WVT'^T e * r
The +bo fold works because softmax rows sum to 1: with unnormalized
e = exp(logits) and r = 1/sum_k e,  r * (WVT+bo)^T e = WVT^T en + bo.

The whole pipeline is DMA-bound (the cost model serializes all DMA traffic
on one shared 360 B/ns device): x-in 16.8MB + out 16.8MB ~= 93us. Everything
else is arranged to fit underneath:
  - out is written fp16 (host upcasts); rel err ~8e-4 vs 2e-2 budget.
  - out matmuls are TRANSPOSED ([pixel, channel] psum, lhsT = e-slice,
    rhs = WVT'): softmax denominators land per-PARTITION, so the
    normalization multiply fuses into the psum->sbuf convert copies
    (ACT activation Copy with scale=r, DVE tensor_scalar_mul) - no
    broadcast matmul, no separate normalize pass.
  - den[n] = sum_k e[k,n] comes from 4 free-size-1 matmuls (lhsT=e chunk,
    rhs=ones column) - negligible PE time.
  - F=512 pixel tiles amortize fixed per-op overheads; x is DMAd in
    1024-column chunks (2KB descriptors) via Pool/SWDGE, out on SP/HWDGE.
Host writes back out^T [N, C] fp16; kernel() transposes + upcasts.

Per-2-tile engine budget (ns, cost model): DMA 2912 | PE ~1750 |
ACT 1836 | DVE ~1450 | Pool ~1170. TimelineSim exec ~ 100us/core.
"""

import numpy as np

import concourse.bacc as bacc
import concourse.mybir as mybir
import concourse.tile as tile
from concourse import bass_utils

F32 = mybir.dt.float32
F32R = mybir.dt.float32r
F16 = mybir.dt.float16

B, C, H, W = 8, 512, 128, 128
N = H * W                    # 16384 pixels per image
CK, CV, K = 256, 256, 64
P = 128                      # SBUF partitions
F = 512                      # pixel-tile width
NT = N // F                  # 32 tiles
FD = 1024                    # x DMA chunk width (2 tiles)
CI_CH = C // P               # 4 contraction chunks over C
Q_CH = CK // P               # 2 chunks over Ck
V_CH = CV // P               # 2 chunks over Cv
NC_CH = F // P               # 4 pixel chunks per tile
SCALE = CK ** -0.5           # 1/16

_CACHED = None


def _build():
    nc = bacc.Bacc("TRN2", target_bir_lowering=False, debug=False)

    X = nc.dram_tensor("x", [C, N], F16, kind="ExternalInput").ap()
    # pack16[c, :] = [pf(64) | wkT(256) | wvT(256)] in fp16
    PACK16 = nc.dram_tensor("pack16", [C, 576], F16, kind="ExternalInput").ap()
    WQ = nc.dram_tensor("wq", [CK, C], F16, kind="ExternalInput").ap()
    WO16 = nc.dram_tensor("wo", [CV, C], F16, kind="ExternalInput").ap()
    # crow = [bk(256) | bv(256) | ones(256) | bo(512)] as one row
    CROW = nc.dram_tensor("crow", [1, 1280], F32, kind="ExternalInput").ap()
    BQS16 = nc.dram_tensor("bqs16", [P, 2], F16, kind="ExternalInput").ap()
    # out^T: row n holds all 512 output channels of pixel n, affine uint8:
    # u8 = rne(out*s + 128). The scale s rides in via host-prescaled Wo/bo
    # (WVT'' = s*WVT'), so the compiled module is identical on every core.
    OUTT = nc.dram_tensor("out", [N, C], mybir.dt.uint8, kind="ExternalOutput").ap()

    x_r = X.rearrange("(co p) n -> p co n", p=P)                   # [128, 4, N]
    out_r = OUTT.rearrange("(t c p) o -> p t c o", c=NC_CH, p=P)   # [128, 32, 4, 512]

    with tile.TileContext(nc) as tc:
        with tc.tile_pool(name="const", bufs=1) as cp:
            # tiny constant rows FIRST: the whole setup-matmul chain waits on
            # crow (bias rows), so it must not queue behind the big weights
            crow = cp.tile([1, 1280], F32R)
            nc.scalar.dma_start(crow, CROW.bitcast(F32R))
            bk_row = crow[:, 0:CK]
            bv_row = crow[:, CK:CK + CV]
            ones_row = crow[:, 512:768]
            bo_row = crow[:, 768:1280]
            bqs = cp.tile([P, 2], F16)
            nc.scalar.dma_start(bqs, BQS16)
            ones_col = cp.tile([K, 1], F16)
            nc.vector.memset(ones_col, 1.0)
            b128 = cp.tile([P, 1], F32)
            nc.vector.memset(b128, 128.0)
            pack = cp.tile([P, CI_CH, 576], F16)
            nc.sync.dma_start(pack, PACK16.rearrange("(co p) q -> p co q", p=P))
            pf = pack[:, :, 0:K]
            wk = pack[:, :, K:K + CK]
            wv = pack[:, :, K + CK:K + CK + CV]
            wq = cp.tile([P, Q_CH, C], F16)
            nc.sync.dma_start(wq, WQ.rearrange("(qo p) c -> p qo c", p=P))
            wo = cp.tile([P, V_CH, C], F16)
            nc.sync.dma_start(wo, WO16.rearrange("(vo p) o -> p vo o", p=P))

            keym = cp.tile([P, Q_CH, K], F16)    # [q-part, q-chunk, k]
            v2sb = cp.tile([P, V_CH, K], F16)    # value[k,v] as [v-part, vo, k]
            wvt = cp.tile([K, C], F16)           # WVT'[k,o] = (Wo@value^T)^T + bo
            msim = cp.tile([P, CI_CH, K], F16)   # M[c,k] = sum_q Wq[q,c]*keym[q,k]
            sbias = cp.tile([K, 1], F32)         # sum_q (bq[q]/16)*keym[q,k]

            # ---- one-time setup: keym, value, WVT', M, sbias
            with tc.tile_pool(name="setup_ps", bufs=1, space="PSUM") as sps:
                kps = sps.tile([P, Q_CH, K], F32)
                for qi in range(Q_CH):
                    for ci in range(CI_CH):
                        nc.tensor.matmul(
                            kps[:, qi, :],
                            wk[:, ci, qi * P:(qi + 1) * P],
                            pf[:, ci, :],
                            start=(ci == 0), stop=False,
                        )
                    # += bk[q] * ones[k]
                    nc.tensor.matmul(
                        kps[:, qi, :],
                        bk_row[:, qi * P:(qi + 1) * P],
                        ones_row[:, :K],
                        start=False, stop=True,
                    )
                nc.vector.tensor_copy(keym, kps)

                v2ps = sps.tile([P, V_CH, K], F32)
                for vi in range(V_CH):
                    for ci in range(CI_CH):
                        nc.tensor.matmul(
                            v2ps[:, vi, :],
                            wv[:, ci, vi * P:(vi + 1) * P],
                            pf[:, ci, :],
                            start=(ci == 0), stop=False,
                        )
                    nc.tensor.matmul(
                        v2ps[:, vi, :],
                        bv_row[:, vi * P:(vi + 1) * P],
                        ones_row[:, :K],
                        start=False, stop=True,
                    )
                nc.vector.tensor_copy(v2sb, v2ps)

                wvtps = sps.tile([K, C], F32)
                for vi in range(V_CH):
                    nc.tensor.matmul(
                        wvtps, v2sb[:, vi, :], wo[:, vi, :],
                        start=(vi == 0), stop=False,
                    )
                # += ones[k] * bo[o]  (valid because softmax rows sum to 1)
                nc.tensor.matmul(
                    wvtps, ones_row[:, :K], bo_row,
                    start=False, stop=True,
                )
                nc.vector.tensor_copy(wvt, wvtps)

                mps = sps.tile([P, CI_CH, K], F32)
                for ci in range(CI_CH):
                    for qi in range(Q_CH):
                        nc.tensor.matmul(
                            mps[:, ci, :],
                            wq[:, qi, ci * P:(ci + 1) * P],
                            keym[:, qi, :],
                            start=(qi == 0), stop=(qi == Q_CH - 1),
                        )
                nc.vector.tensor_copy(msim, mps)

                sbps = sps.tile([K, 1], F32)
                for qi in range(Q_CH):
                    nc.tensor.matmul(
                        sbps, keym[:, qi, :], bqs[:, qi:qi + 1],
                        start=(qi == 0), stop=(qi == Q_CH - 1),
                    )
                nc.vector.tensor_copy(sbias, sbps)

            # ---- steady-state pipeline over 32 tiles of F=512 pixels
            # Software-pipelined one tile deep: iteration t emits the
            # sim/exp front-end for tile t and the den/out/copy/DMA back-end
            # for tile t-1, so no engine's in-order stream ever waits on a
            # same-tile cross-engine chain (ACT: exp(t) precedes copies(t-1)).
            with (
                tc.tile_pool(name="xin", bufs=4) as xp,
                tc.tile_pool(name="esb", bufs=4) as ep,
                tc.tile_pool(name="rsb", bufs=3) as rp,
                tc.tile_pool(name="outsb", bufs=5) as osp,
                tc.tile_pool(name="simps", bufs=2, space="PSUM") as simp,
                tc.tile_pool(name="denps", bufs=2, space="PSUM") as denp,
                tc.tile_pool(name="outps", bufs=4, space="PSUM") as outp,
            ):
                x_t = None
                e_p = None
                for t in range(NT + 1):
                    if t < NT:
                        n0 = t * F
                        if t % 2 == 0:
                            x_t = xp.tile([P, CI_CH, FD], F16, tag="x")
                            nc.gpsimd.dma_start(x_t, x_r[:, :, n0:n0 + FD])
                        off = (t % 2) * F

                        # sim[k, n] = M^T x (Q projection folded into M)
                        sim = simp.tile([K, F], F32, tag="sim")
                        for ci in range(CI_CH):
                            nc.tensor.matmul(
                                sim, msim[:, ci, :], x_t[:, ci, off:off + F],
                                start=(ci == 0), stop=(ci == CI_CH - 1),
                            )
                        e = ep.tile([K, F], F16, tag="e")
                        nc.scalar.activation(
                            e, sim, mybir.ActivationFunctionType.Exp,
                            scale=SCALE, bias=sbias,
                        )

                    if e_p is not None:
                        tp = t - 1
                        # den[n] (pixels on partitions): lhsT = e slice
                        den = denp.tile([P, NC_CH], F32, tag="den")
                        for c in range(NC_CH):
                            nc.tensor.matmul(
                                den[:, c:c + 1],
                                e_p[:, c * P:(c + 1) * P], ones_col,
                                start=True, stop=True,
                            )
                        r_sb = rp.tile([P, NC_CH], F32, tag="r")
                        with nc.allow_low_precision(reason="f32 recip on DVE"):
                            nc.vector.reciprocal(r_sb, den)

                        # out^T chunks [pixel, channel]; normalization (x r)
                        # and affine uint8 quantization (+128, rne) fuse into
                        # the psum->sbuf convert copies
                        out_sb = osp.tile([P, NC_CH, C], mybir.dt.uint8, tag="osb")
                        for c in range(NC_CH):
                            op = outp.tile([P, C], F32, tag="op")
                            nc.tensor.matmul(
                                op, e_p[:, c * P:(c + 1) * P], wvt,
                                start=True, stop=True,
                            )
                            if c < 2:
                                nc.scalar.activation(
                                    out_sb[:, c, :], op,
                                    mybir.ActivationFunctionType.Identity,
                                    bias=b128, scale=r_sb[:, c:c + 1],
                                )
                            else:
                                nc.vector.tensor_scalar(
                                    out_sb[:, c, :], op,
                                    r_sb[:, c:c + 1], 128.0,
                                    op0=mybir.AluOpType.mult,
                                    op1=mybir.AluOpType.add,
                                )
                        nc.sync.dma_start(out_r[:, tp, :, :], out_sb)
                    e_p = e if t < NT else None

    nc.compile()
    return nc


def _get_nc():
    global _CACHED
    if _CACHED is None:
        _CACHED = _build()
    return _CACHED


def kernel(x, proxy, Wq, bq, Wk, bk, Wv, bv, Wo, bo, **run_kwargs):
    nc = _get_nc()

    w16 = np.concatenate(
        [np.asarray(Wk).T, np.asarray(Wv).T], axis=1
    ).astype(np.float16)
    wo32 = np.asarray(Wo, np.float32)
    wv16 = np.asarray(Wv).astype(np.float16).astype(np.float32)
    bo32 = np.asarray(bo, np.float32)
    shared = {
        "wq": np.ascontiguousarray(Wq).astype(np.float16),
        "bqs16": np.ascontiguousarray(
            (np.asarray(bq, np.float32) * SCALE).reshape(2, P).T
        ).astype(np.float16),
    }
    in_maps = []
    scales = []
    for b in range(B):
        m = dict(shared)
        m["x"] = np.ascontiguousarray(x[b]).reshape(C, N).astype(np.float16)
        pf16 = np.asarray(proxy[b, :, :, 0]).astype(np.float16)
        m["pack16"] = np.ascontiguousarray(np.concatenate([pf16, w16], axis=1))
        # uint8 scale: out rows are convex combinations of WVT' columns, so
        # max|WVT' + bo| bounds |out|. Replicate the device's fp16 setup math.
        v216 = (wv16 @ pf16.astype(np.float32)
                + np.asarray(bv, np.float32)[:, None]).astype(np.float16)
        wvtp = wo32 @ v216.astype(np.float32) + bo32[:, None]
        s = 126.0 / (np.abs(wvtp).max() * 1.01)
        scales.append(s)
        crow = np.concatenate(
            [np.asarray(bk, np.float32).reshape(1, CK),
             np.asarray(bv, np.float32).reshape(1, CV),
             np.ones((1, 256), np.float32),
             (s * bo32).reshape(1, C)], axis=1)
        m["crow"] = np.ascontiguousarray(crow)
        m["wo"] = np.ascontiguousarray(s * wo32.T).astype(np.float16)
        in_maps.append(m)

    res = bass_utils.run_bass_kernel_spmd(
        nc, in_maps, core_ids=list(range(B)), **run_kwargs
    )
    out = np.stack(
        [(res.results[b]["out"].astype(np.float32) - 128.0).T / scales[b]
         for b in range(B)], axis=0
    )
    if run_kwargs:
        kernel.last_results = res
    return out.reshape(B, C, H, W)
